# revision 9
# baseline (speedup 1.0000x reference)
"""Trainium2 Bass kernel for the CANN uniaxial-stress model (nn_CANN_81252191306279).

Math
----
Per sample x (stretch), with r = 1/x, z = 1/x^2:
    P1 = h * f,   f = x - z
    h  = 2*C0 + 2*B1*x^2 + 2*Cm1*r + 2*B2*r^3
(w_exp <= 1e-5 linearized exactly; A1,B1,A2,B2,C0,Cm1 folded on host.)

Device mapping (fp16 HBM I/O; h' = h/(2*B2) so the unit r^3 coefficient
fits 3 scalar slots; host multiplies the fp32 output by 2*B2):

  Steady-state tiles (ACT-assisted):
    ACT : l = Ln(x);  z = Exp(-2*l)     (= 1/x^2 to table precision)
    DVE : f = x - z                      stock fp16 tensor_sub (2x mode)
          h' = CANN_H3_ANT(x, z)         fused 7-op custom pass:
               r = z*x; h' = (x^2*c0 + c1) + (z + c2)*r
          P' = f * h'                     stock fp16 tensor_mul (2x mode)

  First tile (DVE-only, hides the ACT warm-up: table load + Ln/Exp of
  tile 0 would otherwise stall the Vector engine ~6.7us at startup):
    DVE : f  = CANN_F_ANT(x)             fused 7-op pass: X = x*x;
               z1 = NR1(bitcast(~X)*s0)  (exponent-flip reciprocal seed +
               one Newton step, (s0,s1) minimax-refit); f = x - z1
          h' = CANN_H_ANT(x, f)          fused 8-op pass: y2 = x - f
               (recovers z1 exactly), r = y2*x, same h' form
          P' = f * h'

Measured rates (NTFF, per 2048 cols): ACT pass 1989ns, custom-DVE pass
2284ns (stock 1x rate), stock fp16 TT 1216ns (2x_1P).  GpSimd measured
2.7x slower at TT and inflates DVE via SBUF port contention - keep idle.
Pipeline: V busy ~36us, ACT ~30us, DMA ~24us, V starts ~8.5us.

Error: ACT-path tiles ~3.2e-3 rel-to-max (fp16 stream rounding), NR-path
tile ~5.7e-3 (numpy bit-level emulation, confirmed exactly by HW runs),
vs the 2e-2 harness gate.

Sharding: pure data parallel, N=2^24 split contiguously across 8 cores
(2,097,152 samples -> [128, 16384] per core), weights folded into immediates.
"""

import os
import sys

for _p in ("/opt/trn_rl_repo",):
    if _p not in sys.path and os.path.isdir(_p):
        sys.path.insert(0, _p)

import numpy as np

N = 16777216
NCORES = 8
P = 128
PER_CORE = N // NCORES           # 2097152
FCOL = PER_CORE // P             # 16384
# (width, style): tile 0 runs DVE-only ("nr"); the rest ACT-assisted ("act")
TILES = [(2048, "nr"), (2048, "act"), (4096, "act"), (4096, "act"),
         (3072, "act"), (1024, "act")]
# minimax-refit (seed-scale, newton-const) for the 1-NR 1/x^2 estimate
S0_NR = -0.23765558
S1_NR = 2.0014041

_CACHE = {}


def _derive_consts(w_identity, w_exp, w_psi):
    wi = np.asarray(w_identity, np.float64).reshape(4)
    we = np.asarray(w_exp, np.float64).reshape(4)
    wp = np.asarray(w_psi, np.float64).reshape(8)
    c0, c1 = wp[0] * wi[0], wp[1] * wi[1]
    c2, c3 = 2 * wp[2] * wi[2], 2 * wp[3] * wi[3]
    a0, a1, a2, a3 = we
    k4, k5 = wp[4] * a0, wp[5] * a1
    k6, k7 = 2 * wp[6] * a2, 2 * wp[7] * a3
    A1, B1 = c0 + k4, c2 + k4 * a0 + k6
    A2, B2 = c1 + k5, c3 + k5 * a1 + k7
    C0 = A1 - 3 * B1 + 2 * B2
    Cm1 = 2 * B1 + A2 - 3 * B2
    return dict(B1=B1, B2=B2, C0=C0, Cm1=Cm1)


def _cpu_fallback(stretch, w_identity, w_exp, w_psi):
    # Degenerate-weight path (B2 ~ 0); exact reference math on host.
    x = np.asarray(stretch, np.float64)
    wi = np.asarray(w_identity, np.float64).reshape(4)
    we = np.asarray(w_exp, np.float64).reshape(4)
    wp = np.asarray(w_psi, np.float64).reshape(8)
    I1 = x * x + 2.0 / x
    I2 = 2.0 * x + 1.0 / (x * x)
    x1, x2 = I1 - 3.0, I2 - 3.0
    d1 = wp[0] * wi[0] + 2 * wp[2] * wi[2] * x1 \
        + wp[4] * we[0] * np.exp(we[0] * x1) \
        + 2 * wp[6] * we[2] * x1 * np.exp(we[2] * x1 * x1)
    d2 = wp[1] * wi[1] + 2 * wp[3] * wi[3] * x2 \
        + wp[5] * we[1] * np.exp(we[1] * x2) \
        + 2 * wp[7] * we[3] * x2 * np.exp(we[3] * x2 * x2)
    P1 = 2.0 * (d1 + d2 / x) * (x - 1.0 / (x * x))
    return P1.astype(np.float32)


def _register_dve_ops():
    """Register the three fused ops in dve_ops' catalog (append-only, rows
    17-19 of the 31 available). Idempotent."""
    import concourse.dve_ops as dve_ops
    have = {op.name: op for op in dve_ops.OPS}
    want = ("CANN_F_ANT", "CANN_H_ANT", "CANN_H3_ANT")
    if all(n in have for n in want):
        return tuple(have[n] for n in want)

    from concourse.dve_spec import (
        Spec, Src0, Src1, C0, C1, C2, AluOp, Bin, lower, _has_src1,
    )
    from concourse.dve_uop import DveOpSpec

    def _f_ref(in0, in1, s0, s1, imm2):
        x = in0.astype(np.float32)
        x2 = x * x
        nX = (~x2.view(np.int32)).view(np.float32)
        z0 = nX * np.float32(s0)
        z1 = z0 * (np.float32(s1) - x2 * z0)
        return x - z1

    def _h_ref(in0, in1, s0, s1, imm2):
        x = in0.astype(np.float32)
        f = in1.astype(np.float32)
        y2 = x - f
        r = y2 * x
        return (x * x * np.float32(s0) + np.float32(s1)) \
            + (y2 + np.float32(imm2)) * r

    def _h3_ref(in0, in1, s0, s1, imm2):
        x = in0.astype(np.float32)
        z = in1.astype(np.float32)
        r = z * x
        return (x * x * np.float32(s0) + np.float32(s1)) \
            + (z + np.float32(imm2)) * r

    _x2 = Src0 * Src0
    _nX = Bin(AluOp.BITWISE_NOT, _x2, _x2)
    _z0 = _nX * C0
    _z1 = _z0 * (C1 - _x2 * _z0)
    f_spec = Spec(body=Src0 - _z1, reference=_f_ref)

    _y2 = Src0 - Src1
    h_spec = Spec(
        body=((Src0 * Src0) * C0 + C1) + (_y2 + C2) * (_y2 * Src0),
        reference=_h_ref)

    h3_spec = Spec(
        body=((Src0 * Src0) * C0 + C1) + (Src1 + C2) * (Src1 * Src0),
        reference=_h3_ref)

    made = []
    for name, spec in (("CANN_F_ANT", f_spec), ("CANN_H_ANT", h_spec),
                       ("CANN_H3_ANT", h3_spec)):
        if name in have:
            made.append(have[name])
            continue
        row = max(dve_ops._SUB_OPCODE_FOR_NAME.values()) + 1
        assert row < 0x20, "custom-DVE row field overflow"
        shas = {}
        for ver in ("v3", "v4"):
            uops = lower(spec, ver=ver)
            shas[ver] = DveOpSpec(
                name=name, opcode=row, uops=uops, rd1_en=_has_src1(spec)
            ).sha(ver)
        dve_ops._SUB_OPCODE_FOR_NAME[name] = row
        op = dve_ops.DveOp(name, spec, subdim=False, uops_sha=shas)
        dve_ops.OPS.append(op)
        dve_ops.CUSTOM_DVE_SPECS[name] = spec
        made.append(op)
    return tuple(made)


def _build_program(consts, precise):
    import concourse.bacc as bacc
    import concourse.mybir as mybir
    import concourse.tile as tile

    # Ln and Exp both live in the natural_log_exp_and_others ACT table set;
    # pin it so walrus's greedy per-function set choice doesn't thrash
    # ACT_TABLE_LOADs (~2.6us each).
    if not getattr(bacc, "_act_tables_pinned", False):
        _orig_gat = bacc.get_activation_tables

        def _pinned(arch):
            full = _orig_gat(arch)
            keep = "natural_log_exp_and_others"
            return {n: (fns if n == keep else set()) for n, fns in full.items()}

        bacc.get_activation_tables = _pinned
        bacc._act_tables_pinned = True

    f_op, h_op, h3_op = _register_dve_ops()

    f16 = mybir.dt.float16
    f32 = mybir.dt.float32
    Ln = mybir.ActivationFunctionType.Ln
    Exp = mybir.ActivationFunctionType.Exp
    cc0 = float(np.float32(consts["B1"] / consts["B2"]))
    cc1 = float(np.float32(consts["C0"] / consts["B2"]))
    cc2 = float(np.float32(consts["Cm1"] / consts["B2"]))

    nc = bacc.Bacc("TRN2", target_bir_lowering=False, debug=False)

    x_ap = nc.dram_tensor("x", [P, FCOL], f16, kind="ExternalInput").ap()
    o_ap = nc.dram_tensor("o", [P, FCOL], f16, kind="ExternalOutput").ap()

    with tile.TileContext(nc) as tc:
        with (
            tc.tile_pool(name="xin", bufs=5) as px,
            tc.tile_pool(name="lpl", bufs=3) as pl,
            tc.tile_pool(name="zpl", bufs=4) as pz,
            tc.tile_pool(name="fpl", bufs=3) as pf,
            tc.tile_pool(name="hpl", bufs=3) as ph,
        ):
            assert sum(w for w, _ in TILES) == FCOL
            off = 0
            for FD_i, style in TILES:
                cs = slice(off, off + FD_i)
                off += FD_i
                tx = px.tile([P, FD_i], f16, tag="tx")
                nc.sync.dma_start(out=tx[:], in_=x_ap[:, cs])

                tf = pf.tile([P, FD_i], f16, tag="tf")
                th = ph.tile([P, FD_i], f16, tag="th")
                if style == "nr":
                    nc.vector._custom_dve(
                        f_op, out=tf[:], in0=tx[:], s0=S0_NR, s1=S1_NR)
                    nc.vector._custom_dve(
                        h_op, out=th[:], in0=tx[:], in1=tf[:],
                        s0=cc0, s1=cc1, imm2=cc2)
                else:
                    tl = pl.tile([P, FD_i], f32, tag="tl")
                    nc.scalar.activation(tl[:], tx[:], Ln, bias=0.0, scale=1.0)
                    tz = pz.tile([P, FD_i], f16, tag="tz")
                    nc.scalar.activation(tz[:], tl[:], Exp, bias=0.0,
                                         scale=-2.0)
                    nc.vector.tensor_sub(tf[:], tx[:], tz[:])
                    nc.vector._custom_dve(
                        h3_op, out=th[:], in0=tx[:], in1=tz[:],
                        s0=cc0, s1=cc1, imm2=cc2)

                # P' = f * h', in place over h'
                nc.vector.tensor_mul(th[:], tf[:], th[:])

                nc.sync.dma_start(out=o_ap[:, cs], in_=th[:])

    nc.compile()
    return nc


def _run(stretch, w_identity, w_exp, w_psi, precise=False, trace=False):
    from concourse.bass_utils import run_bass_kernel_spmd

    x = np.ascontiguousarray(np.asarray(stretch, dtype=np.float32))
    assert x.shape == (N,), x.shape
    consts = _derive_consts(w_identity, w_exp, w_psi)
    if not (np.isfinite(list(consts.values())).all()
            and consts["B2"] > 1e-12):
        return _cpu_fallback(stretch, w_identity, w_exp, w_psi), None

    key = (tuple(sorted(consts.items())), precise)
    if key not in _CACHE:
        _CACHE[key] = _build_program(consts, precise)
    nc = _CACHE[key]

    xs = x.astype(np.float16).reshape(NCORES, P, FCOL)
    in_maps = [{"x": xs[i]} for i in range(NCORES)]
    res = run_bass_kernel_spmd(nc, in_maps, list(range(NCORES)), trace=trace)
    scale = np.float32(2.0 * consts["B2"])
    out = np.concatenate(
        [np.asarray(res.results[i]["o"]).astype(np.float32).reshape(-1)
         for i in range(NCORES)]) * scale
    return out.astype(np.float32), res


def kernel(stretch, w_identity, w_exp, w_psi):
    out, _ = _run(stretch, w_identity, w_exp, w_psi)
    return out


# revision 10
# speedup vs baseline: 1.0122x; 1.0122x over previous
"""Trainium2 Bass kernel for the CANN uniaxial-stress model (nn_CANN_81252191306279).

Math
----
Per sample x (stretch), with r = 1/x, z = 1/x^2:
    P1 = h * f,   f = x - z
    h  = 2*C0 + 2*B1*x^2 + 2*Cm1*r + 2*B2*r^3
(w_exp <= 1e-5 linearized exactly; A1,B1,A2,B2,C0,Cm1 folded on host.)

Device mapping (fp16 HBM I/O; h' = h/(2*B2) so the unit r^3 coefficient
fits 3 scalar slots; host multiplies the fp32 output by 2*B2):

  Steady-state tiles (ACT-assisted):
    ACT : l = Ln(x);  z = Exp(-2*l)     (= 1/x^2 to table precision)
    DVE : f = x - z                      stock fp16 tensor_sub (2x mode)
          h' = CANN_H3_ANT(x, z)         fused 7-op custom pass:
               r = z*x; h' = (x^2*c0 + c1) + (z + c2)*r
          P' = f * h'                     stock fp16 tensor_mul (2x mode)

  First tile (DVE-only, hides the ACT warm-up: table load + Ln/Exp of
  tile 0 would otherwise stall the Vector engine ~6.7us at startup):
    DVE : f  = CANN_F_ANT(x)             fused 7-op pass: X = x*x;
               z1 = NR1(bitcast(~X)*s0)  (exponent-flip reciprocal seed +
               one Newton step, (s0,s1) minimax-refit); f = x - z1
          h' = CANN_H_ANT(x, f)          fused 8-op pass: y2 = x - f
               (recovers z1 exactly), r = y2*x, same h' form
          P' = f * h'

Measured rates (NTFF, per 2048 cols): ACT pass 1989ns, custom-DVE pass
2284ns (stock 1x rate), stock fp16 TT 1216ns (2x_1P).  GpSimd measured
2.7x slower at TT and inflates DVE via SBUF port contention - keep idle.
Pipeline: V busy ~36us, ACT ~30us, DMA ~24us, V starts ~8.5us.

Error: ACT-path tiles ~3.2e-3 rel-to-max (fp16 stream rounding), NR-path
tile ~5.7e-3 (numpy bit-level emulation, confirmed exactly by HW runs),
vs the 2e-2 harness gate.

Sharding: pure data parallel, N=2^24 split contiguously across 8 cores
(2,097,152 samples -> [128, 16384] per core), weights folded into immediates.
"""

import os
import sys

for _p in ("/opt/trn_rl_repo",):
    if _p not in sys.path and os.path.isdir(_p):
        sys.path.insert(0, _p)

import numpy as np

N = 16777216
NCORES = 8
P = 128
PER_CORE = N // NCORES           # 2097152
FCOL = PER_CORE // P             # 16384
# (width, style): tile 0 runs DVE-only ("nr"); the rest ACT-assisted ("act")
TILES = [(2048, "nr"), (2048, "act"), (4096, "act"), (4096, "act"),
         (3072, "act"), (1024, "act")]
# minimax-refit (seed-scale, newton-const) for the 1-NR 1/x^2 estimate
S0_NR = -0.23765558
S1_NR = 2.0014041

_CACHE = {}


def _derive_consts(w_identity, w_exp, w_psi):
    wi = np.asarray(w_identity, np.float64).reshape(4)
    we = np.asarray(w_exp, np.float64).reshape(4)
    wp = np.asarray(w_psi, np.float64).reshape(8)
    c0, c1 = wp[0] * wi[0], wp[1] * wi[1]
    c2, c3 = 2 * wp[2] * wi[2], 2 * wp[3] * wi[3]
    a0, a1, a2, a3 = we
    k4, k5 = wp[4] * a0, wp[5] * a1
    k6, k7 = 2 * wp[6] * a2, 2 * wp[7] * a3
    A1, B1 = c0 + k4, c2 + k4 * a0 + k6
    A2, B2 = c1 + k5, c3 + k5 * a1 + k7
    C0 = A1 - 3 * B1 + 2 * B2
    Cm1 = 2 * B1 + A2 - 3 * B2
    return dict(B1=B1, B2=B2, C0=C0, Cm1=Cm1)


def _cpu_fallback(stretch, w_identity, w_exp, w_psi):
    # Degenerate-weight path (B2 ~ 0); exact reference math on host.
    x = np.asarray(stretch, np.float64)
    wi = np.asarray(w_identity, np.float64).reshape(4)
    we = np.asarray(w_exp, np.float64).reshape(4)
    wp = np.asarray(w_psi, np.float64).reshape(8)
    I1 = x * x + 2.0 / x
    I2 = 2.0 * x + 1.0 / (x * x)
    x1, x2 = I1 - 3.0, I2 - 3.0
    d1 = wp[0] * wi[0] + 2 * wp[2] * wi[2] * x1 \
        + wp[4] * we[0] * np.exp(we[0] * x1) \
        + 2 * wp[6] * we[2] * x1 * np.exp(we[2] * x1 * x1)
    d2 = wp[1] * wi[1] + 2 * wp[3] * wi[3] * x2 \
        + wp[5] * we[1] * np.exp(we[1] * x2) \
        + 2 * wp[7] * we[3] * x2 * np.exp(we[3] * x2 * x2)
    P1 = 2.0 * (d1 + d2 / x) * (x - 1.0 / (x * x))
    return P1.astype(np.float32)


def _register_dve_ops():
    """Register the three fused ops in dve_ops' catalog (append-only, rows
    17-19 of the 31 available). Idempotent."""
    import concourse.dve_ops as dve_ops
    have = {op.name: op for op in dve_ops.OPS}
    want = ("CANN_F_ANT", "CANN_H_ANT", "CANN_H3_ANT")
    if all(n in have for n in want):
        return tuple(have[n] for n in want)

    from concourse.dve_spec import (
        Spec, Src0, Src1, C0, C1, C2, AluOp, Bin, lower, _has_src1,
    )
    from concourse.dve_uop import DveOpSpec

    def _f_ref(in0, in1, s0, s1, imm2):
        x = in0.astype(np.float32)
        x2 = x * x
        nX = (~x2.view(np.int32)).view(np.float32)
        z0 = nX * np.float32(s0)
        z1 = z0 * (np.float32(s1) - x2 * z0)
        return x - z1

    def _h_ref(in0, in1, s0, s1, imm2):
        x = in0.astype(np.float32)
        f = in1.astype(np.float32)
        y2 = x - f
        r = y2 * x
        return (x * x * np.float32(s0) + np.float32(s1)) \
            + (y2 + np.float32(imm2)) * r

    def _h3_ref(in0, in1, s0, s1, imm2):
        x = in0.astype(np.float32)
        z = in1.astype(np.float32)
        r = z * x
        return (x * x * np.float32(s0) + np.float32(s1)) \
            + (z + np.float32(imm2)) * r

    _x2 = Src0 * Src0
    _nX = Bin(AluOp.BITWISE_NOT, _x2, _x2)
    _z0 = _nX * C0
    _z1 = _z0 * (C1 - _x2 * _z0)
    f_spec = Spec(body=Src0 - _z1, reference=_f_ref)

    _y2 = Src0 - Src1
    h_spec = Spec(
        body=((Src0 * Src0) * C0 + C1) + (_y2 + C2) * (_y2 * Src0),
        reference=_h_ref)

    h3_spec = Spec(
        body=((Src0 * Src0) * C0 + C1) + (Src1 + C2) * (Src1 * Src0),
        reference=_h3_ref)

    made = []
    for name, spec in (("CANN_F_ANT", f_spec), ("CANN_H_ANT", h_spec),
                       ("CANN_H3_ANT", h3_spec)):
        if name in have:
            made.append(have[name])
            continue
        row = max(dve_ops._SUB_OPCODE_FOR_NAME.values()) + 1
        assert row < 0x20, "custom-DVE row field overflow"
        shas = {}
        for ver in ("v3", "v4"):
            uops = lower(spec, ver=ver)
            shas[ver] = DveOpSpec(
                name=name, opcode=row, uops=uops, rd1_en=_has_src1(spec)
            ).sha(ver)
        dve_ops._SUB_OPCODE_FOR_NAME[name] = row
        op = dve_ops.DveOp(name, spec, subdim=False, uops_sha=shas)
        dve_ops.OPS.append(op)
        dve_ops.CUSTOM_DVE_SPECS[name] = spec
        made.append(op)
    return tuple(made)


def _build_program(consts, precise):
    import concourse.bacc as bacc
    import concourse.mybir as mybir
    import concourse.tile as tile

    # Ln and Exp both live in the natural_log_exp_and_others ACT table set;
    # pin it so walrus's greedy per-function set choice doesn't thrash
    # ACT_TABLE_LOADs (~2.6us each).
    if not getattr(bacc, "_act_tables_pinned", False):
        _orig_gat = bacc.get_activation_tables

        def _pinned(arch):
            full = _orig_gat(arch)
            keep = "natural_log_exp_and_others"
            return {n: (fns if n == keep else set()) for n, fns in full.items()}

        bacc.get_activation_tables = _pinned
        bacc._act_tables_pinned = True

    f_op, h_op, h3_op = _register_dve_ops()

    f16 = mybir.dt.float16
    f32 = mybir.dt.float32
    Ln = mybir.ActivationFunctionType.Ln
    Exp = mybir.ActivationFunctionType.Exp
    cc0 = float(np.float32(consts["B1"] / consts["B2"]))
    cc1 = float(np.float32(consts["C0"] / consts["B2"]))
    cc2 = float(np.float32(consts["Cm1"] / consts["B2"]))

    nc = bacc.Bacc("TRN2", target_bir_lowering=False, debug=False)

    x_ap = nc.dram_tensor("x", [P, FCOL], f16, kind="ExternalInput").ap()
    o_ap = nc.dram_tensor("o", [P, FCOL], f16, kind="ExternalOutput").ap()

    with tile.TileContext(nc) as tc:
        with (
            tc.tile_pool(name="xin", bufs=5) as px,
            tc.tile_pool(name="lpl", bufs=1) as pl,
            tc.tile_pool(name="zpl", bufs=4) as pz,
            tc.tile_pool(name="fpl", bufs=3) as pf,
            tc.tile_pool(name="hpl", bufs=3) as ph,
        ):
            assert sum(w for w, _ in TILES) == FCOL
            off = 0
            for FD_i, style in TILES:
                cs = slice(off, off + FD_i)
                off += FD_i
                tx = px.tile([P, FD_i], f16, tag="tx")
                nc.sync.dma_start(out=tx[:], in_=x_ap[:, cs])

                tf = pf.tile([P, FD_i], f16, tag="tf")
                th = ph.tile([P, FD_i], f16, tag="th")
                if style == "nr":
                    nc.vector._custom_dve(
                        f_op, out=tf[:], in0=tx[:], s0=S0_NR, s1=S1_NR)
                    nc.vector._custom_dve(
                        h_op, out=th[:], in0=tx[:], in1=tf[:],
                        s0=cc0, s1=cc1, imm2=cc2)
                else:
                    tl = pl.tile([P, FD_i], f32, tag="tl")
                    nc.scalar.activation(tl[:], tx[:], Ln, bias=0.0, scale=1.0)
                    tz = pz.tile([P, FD_i], f16, tag="tz")
                    nc.scalar.activation(tz[:], tl[:], Exp, bias=0.0,
                                         scale=-2.0)
                    nc.vector.tensor_sub(tf[:], tx[:], tz[:])
                    nc.vector._custom_dve(
                        h3_op, out=th[:], in0=tx[:], in1=tz[:],
                        s0=cc0, s1=cc1, imm2=cc2)

                # P' = f * h', in place over h'
                nc.vector.tensor_mul(th[:], tf[:], th[:])

                nc.sync.dma_start(out=o_ap[:, cs], in_=th[:])

    nc.compile()
    return nc


def _run(stretch, w_identity, w_exp, w_psi, precise=False, trace=False):
    from concourse.bass_utils import run_bass_kernel_spmd

    x = np.ascontiguousarray(np.asarray(stretch, dtype=np.float32))
    assert x.shape == (N,), x.shape
    consts = _derive_consts(w_identity, w_exp, w_psi)
    if not (np.isfinite(list(consts.values())).all()
            and consts["B2"] > 1e-12):
        return _cpu_fallback(stretch, w_identity, w_exp, w_psi), None

    key = (tuple(sorted(consts.items())), precise)
    if key not in _CACHE:
        _CACHE[key] = _build_program(consts, precise)
    nc = _CACHE[key]

    xs = x.astype(np.float16).reshape(NCORES, P, FCOL)
    in_maps = [{"x": xs[i]} for i in range(NCORES)]
    res = run_bass_kernel_spmd(nc, in_maps, list(range(NCORES)), trace=trace)
    scale = np.float32(2.0 * consts["B2"])
    out = np.concatenate(
        [np.asarray(res.results[i]["o"]).astype(np.float32).reshape(-1)
         for i in range(NCORES)]) * scale
    return out.astype(np.float32), res


def kernel(stretch, w_identity, w_exp, w_psi):
    out, _ = _run(stretch, w_identity, w_exp, w_psi)
    return out


# revision 11
# speedup vs baseline: 1.0507x; 1.0380x over previous
"""Trainium2 Bass kernel for the CANN uniaxial-stress model (nn_CANN_81252191306279).

Math
----
Per sample x (stretch), with r = 1/x, z = 1/x^2:
    P1 = h * f,   f = x - z
    h  = 2*C0 + 2*B1*x^2 + 2*Cm1*r + 2*B2*r^3
(w_exp <= 1e-5 linearized exactly; A1,B1,A2,B2,C0,Cm1 folded on host.)

Device mapping (fp16 HBM I/O; h' = h/(2*B2) so the unit r^3 coefficient
fits 3 scalar slots; host multiplies the fp32 output by 2*B2):

  Steady-state tiles (ACT-assisted):
    ACT : l = Ln(x);  z = Exp(-2*l)     (= 1/x^2 to table precision)
    DVE : f = x - z                      stock fp16 tensor_sub (2x mode)
          h' = CANN_H3_ANT(x, z)         fused 7-op custom pass:
               r = z*x; h' = (x^2*c0 + c1) + (z + c2)*r
          P' = f * h'                     stock fp16 tensor_mul (2x mode)

  First tile (DVE-only, hides the ACT warm-up: table load + Ln/Exp of
  tile 0 would otherwise stall the Vector engine ~6.7us at startup):
    DVE : f  = CANN_F_ANT(x)             fused 7-op pass: X = x*x;
               z1 = NR1(bitcast(~X)*s0)  (exponent-flip reciprocal seed +
               one Newton step, (s0,s1) minimax-refit); f = x - z1
          h' = CANN_H_ANT(x, f)          fused 8-op pass: y2 = x - f
               (recovers z1 exactly), r = y2*x, same h' form
          P' = f * h'

Measured rates (NTFF, per 2048 cols): ACT pass 1989ns, custom-DVE pass
2284ns (stock 1x rate), stock fp16 TT 1216ns (2x_1P).  GpSimd measured
2.7x slower at TT and inflates DVE via SBUF port contention - keep idle.
Pipeline: V busy ~36us, ACT ~30us, DMA ~24us, V starts ~8.5us.

Error: ACT-path tiles ~3.2e-3 rel-to-max (fp16 stream rounding), NR-path
tile ~5.7e-3 (numpy bit-level emulation, confirmed exactly by HW runs),
vs the 2e-2 harness gate.

Sharding: pure data parallel, N=2^24 split contiguously across 8 cores
(2,097,152 samples -> [128, 16384] per core), weights folded into immediates.
"""

import os
import sys

for _p in ("/opt/trn_rl_repo",):
    if _p not in sys.path and os.path.isdir(_p):
        sys.path.insert(0, _p)

import numpy as np

N = 16777216
NCORES = 8
P = 128
PER_CORE = N // NCORES           # 2097152
FCOL = PER_CORE // P             # 16384
# (width, style): tile 0 runs DVE-only ("nr"); the rest ACT-assisted ("act")
TILES = [(2048, "nr"), (2048, "act"), (4096, "act"), (4096, "act"),
         (3072, "act"), (1024, "act")]
# minimax-refit (seed-scale, newton-const) for the 1-NR 1/x^2 estimate
S0_NR = -0.23765558
S1_NR = 2.0014041

_CACHE = {}


def _derive_consts(w_identity, w_exp, w_psi):
    wi = np.asarray(w_identity, np.float64).reshape(4)
    we = np.asarray(w_exp, np.float64).reshape(4)
    wp = np.asarray(w_psi, np.float64).reshape(8)
    c0, c1 = wp[0] * wi[0], wp[1] * wi[1]
    c2, c3 = 2 * wp[2] * wi[2], 2 * wp[3] * wi[3]
    a0, a1, a2, a3 = we
    k4, k5 = wp[4] * a0, wp[5] * a1
    k6, k7 = 2 * wp[6] * a2, 2 * wp[7] * a3
    A1, B1 = c0 + k4, c2 + k4 * a0 + k6
    A2, B2 = c1 + k5, c3 + k5 * a1 + k7
    C0 = A1 - 3 * B1 + 2 * B2
    Cm1 = 2 * B1 + A2 - 3 * B2
    return dict(B1=B1, B2=B2, C0=C0, Cm1=Cm1)


def _cpu_fallback(stretch, w_identity, w_exp, w_psi):
    # Degenerate-weight path (B2 ~ 0); exact reference math on host.
    x = np.asarray(stretch, np.float64)
    wi = np.asarray(w_identity, np.float64).reshape(4)
    we = np.asarray(w_exp, np.float64).reshape(4)
    wp = np.asarray(w_psi, np.float64).reshape(8)
    I1 = x * x + 2.0 / x
    I2 = 2.0 * x + 1.0 / (x * x)
    x1, x2 = I1 - 3.0, I2 - 3.0
    d1 = wp[0] * wi[0] + 2 * wp[2] * wi[2] * x1 \
        + wp[4] * we[0] * np.exp(we[0] * x1) \
        + 2 * wp[6] * we[2] * x1 * np.exp(we[2] * x1 * x1)
    d2 = wp[1] * wi[1] + 2 * wp[3] * wi[3] * x2 \
        + wp[5] * we[1] * np.exp(we[1] * x2) \
        + 2 * wp[7] * we[3] * x2 * np.exp(we[3] * x2 * x2)
    P1 = 2.0 * (d1 + d2 / x) * (x - 1.0 / (x * x))
    return P1.astype(np.float32)


def _register_dve_ops():
    """Register the three fused ops in dve_ops' catalog (append-only, rows
    17-19 of the 31 available). Idempotent."""
    import concourse.dve_ops as dve_ops
    have = {op.name: op for op in dve_ops.OPS}
    want = ("CANN_F_ANT", "CANN_H_ANT", "CANN_H3_ANT")
    if all(n in have for n in want):
        return tuple(have[n] for n in want)

    from concourse.dve_spec import (
        Spec, Src0, Src1, C0, C1, C2, AluOp, Bin, lower, _has_src1,
    )
    from concourse.dve_uop import DveOpSpec

    def _f_ref(in0, in1, s0, s1, imm2):
        x = in0.astype(np.float32)
        x2 = x * x
        nX = (~x2.view(np.int32)).view(np.float32)
        z0 = nX * np.float32(s0)
        z1 = z0 * (np.float32(s1) - x2 * z0)
        return x - z1

    def _h_ref(in0, in1, s0, s1, imm2):
        x = in0.astype(np.float32)
        f = in1.astype(np.float32)
        y2 = x - f
        r = y2 * x
        return (x * x * np.float32(s0) + np.float32(s1)) \
            + (y2 + np.float32(imm2)) * r

    def _h3_ref(in0, in1, s0, s1, imm2):
        x = in0.astype(np.float32)
        z = in1.astype(np.float32)
        r = z * x
        return (x * x * np.float32(s0) + np.float32(s1)) \
            + (z + np.float32(imm2)) * r

    _x2 = Src0 * Src0
    _nX = Bin(AluOp.BITWISE_NOT, _x2, _x2)
    _z0 = _nX * C0
    _z1 = _z0 * (C1 - _x2 * _z0)
    f_spec = Spec(body=Src0 - _z1, reference=_f_ref)

    _y2 = Src0 - Src1
    h_spec = Spec(
        body=((Src0 * Src0) * C0 + C1) + (_y2 + C2) * (_y2 * Src0),
        reference=_h_ref)

    h3_spec = Spec(
        body=((Src0 * Src0) * C0 + C1) + (Src1 + C2) * (Src1 * Src0),
        reference=_h3_ref)

    made = []
    for name, spec in (("CANN_F_ANT", f_spec), ("CANN_H_ANT", h_spec),
                       ("CANN_H3_ANT", h3_spec)):
        if name in have:
            made.append(have[name])
            continue
        row = max(dve_ops._SUB_OPCODE_FOR_NAME.values()) + 1
        assert row < 0x20, "custom-DVE row field overflow"
        shas = {}
        for ver in ("v3", "v4"):
            uops = lower(spec, ver=ver)
            shas[ver] = DveOpSpec(
                name=name, opcode=row, uops=uops, rd1_en=_has_src1(spec)
            ).sha(ver)
        dve_ops._SUB_OPCODE_FOR_NAME[name] = row
        op = dve_ops.DveOp(name, spec, subdim=False, uops_sha=shas)
        dve_ops.OPS.append(op)
        dve_ops.CUSTOM_DVE_SPECS[name] = spec
        made.append(op)
    return tuple(made)


def _build_program(consts, precise):
    import concourse.bacc as bacc
    import concourse.mybir as mybir
    import concourse.tile as tile

    # Ln and Exp both live in the natural_log_exp_and_others ACT table set;
    # pin it so walrus's greedy per-function set choice doesn't thrash
    # ACT_TABLE_LOADs (~2.6us each).
    if not getattr(bacc, "_act_tables_pinned", False):
        _orig_gat = bacc.get_activation_tables

        def _pinned(arch):
            full = _orig_gat(arch)
            keep = "natural_log_exp_and_others"
            return {n: (fns if n == keep else set()) for n, fns in full.items()}

        bacc.get_activation_tables = _pinned
        bacc._act_tables_pinned = True

    f_op, h_op, h3_op = _register_dve_ops()

    f16 = mybir.dt.float16
    f32 = mybir.dt.float32
    Ln = mybir.ActivationFunctionType.Ln
    Exp = mybir.ActivationFunctionType.Exp
    cc0 = float(np.float32(consts["B1"] / consts["B2"]))
    cc1 = float(np.float32(consts["C0"] / consts["B2"]))
    cc2 = float(np.float32(consts["Cm1"] / consts["B2"]))

    nc = bacc.Bacc("TRN2", target_bir_lowering=False, debug=False)

    x_ap = nc.dram_tensor("x", [P, FCOL], f16, kind="ExternalInput").ap()
    o_ap = nc.dram_tensor("o", [P, FCOL], f16, kind="ExternalOutput").ap()

    with tile.TileContext(nc) as tc:
        with (
            tc.tile_pool(name="xin", bufs=5) as px,
            tc.tile_pool(name="lpl", bufs=1) as pl,
            tc.tile_pool(name="zpl", bufs=4) as pz,
            tc.tile_pool(name="fpl", bufs=3) as pf,
            tc.tile_pool(name="hpl", bufs=3) as ph,
        ):
            assert sum(w for w, _ in TILES) == FCOL
            # Pre-issue the input DMAs with the first ACT-style tile ahead of
            # the NR tile: ACT's first Ln then starts as soon as its table is
            # loaded instead of waiting behind the NR tile's transfer.
            offs, txs = [], {}
            off = 0
            for i, (FD_i, _style) in enumerate(TILES):
                offs.append(off)
                off += FD_i
            dma_order = [1, 0] + list(range(2, len(TILES)))
            for i in dma_order:
                FD_i, _style = TILES[i]
                cs = slice(offs[i], offs[i] + FD_i)
                tx = px.tile([P, FD_i], f16, tag="tx")
                nc.sync.dma_start(out=tx[:], in_=x_ap[:, cs])
                txs[i] = tx
            for i, (FD_i, style) in enumerate(TILES):
                cs = slice(offs[i], offs[i] + FD_i)
                tx = txs[i]

                tf = pf.tile([P, FD_i], f16, tag="tf")
                th = ph.tile([P, FD_i], f16, tag="th")
                if style == "nr":
                    nc.vector._custom_dve(
                        f_op, out=tf[:], in0=tx[:], s0=S0_NR, s1=S1_NR)
                    nc.vector._custom_dve(
                        h_op, out=th[:], in0=tx[:], in1=tf[:],
                        s0=cc0, s1=cc1, imm2=cc2)
                else:
                    tl = pl.tile([P, FD_i], f32, tag="tl")
                    nc.scalar.activation(tl[:], tx[:], Ln, bias=0.0, scale=1.0)
                    tz = pz.tile([P, FD_i], f16, tag="tz")
                    nc.scalar.activation(tz[:], tl[:], Exp, bias=0.0,
                                         scale=-2.0)
                    nc.vector.tensor_sub(tf[:], tx[:], tz[:])
                    nc.vector._custom_dve(
                        h3_op, out=th[:], in0=tx[:], in1=tz[:],
                        s0=cc0, s1=cc1, imm2=cc2)

                # P' = f * h', in place over h'
                nc.vector.tensor_mul(th[:], tf[:], th[:])

                nc.sync.dma_start(out=o_ap[:, cs], in_=th[:])

    nc.compile()
    return nc


def _run(stretch, w_identity, w_exp, w_psi, precise=False, trace=False):
    from concourse.bass_utils import run_bass_kernel_spmd

    x = np.ascontiguousarray(np.asarray(stretch, dtype=np.float32))
    assert x.shape == (N,), x.shape
    consts = _derive_consts(w_identity, w_exp, w_psi)
    if not (np.isfinite(list(consts.values())).all()
            and consts["B2"] > 1e-12):
        return _cpu_fallback(stretch, w_identity, w_exp, w_psi), None

    key = (tuple(sorted(consts.items())), precise)
    if key not in _CACHE:
        _CACHE[key] = _build_program(consts, precise)
    nc = _CACHE[key]

    xs = x.astype(np.float16).reshape(NCORES, P, FCOL)
    in_maps = [{"x": xs[i]} for i in range(NCORES)]
    res = run_bass_kernel_spmd(nc, in_maps, list(range(NCORES)), trace=trace)
    scale = np.float32(2.0 * consts["B2"])
    out = np.concatenate(
        [np.asarray(res.results[i]["o"]).astype(np.float32).reshape(-1)
         for i in range(NCORES)]) * scale
    return out.astype(np.float32), res


def kernel(stretch, w_identity, w_exp, w_psi):
    out, _ = _run(stretch, w_identity, w_exp, w_psi)
    return out
